# revision 19
# baseline (speedup 1.0000x reference)
"""Channel-attention kernel for Trainium2 (Bass/Tile), 8-core data parallel.

Computes, per batch sample b (x: [B=8, H=128, W=128, C=256] fp32):
    a   = x[b].reshape(N=16384, C)
    G   = a^T @ a                      # [C, C]
    att = softmax(G, axis=-1)
    out = x[b] + beta * (a @ att^T)

Sharding: pure data parallel, one sample per NeuronCore (B == n_cores == 8).

v2 design (vs v1): the residual add is fused into the second matmul by
using M = I + beta * softmax(G)^T as the rhs, so phase 2 is pure matmul
with no elementwise residual pass and no fp32 x residency.  The bf16
rounding of x through the a @ I path is ~2^-9 relative, far inside the
2e-2 gate.  Transposes of a are split 50/50 between the phases (even
chunks in phase 1, odd chunks in phase 2) so phase-1 tensor work stays
under the input-DMA streaming rate and G completes right behind the
last input granule.

Per-core pipeline (N=16384 rows as 128 chunks of 128 rows, in the
partition-contiguous permutation so every HBM DMA is one contiguous run
per partition):
  Phase 1: stream x granules into a small landing pool, cast each
           granule to a resident bf16 buffer, accumulate
           G = sum_i xb_i^T @ xb_i in 2 PSUM banks, PE-transpose the
           even chunks into resident a^T (scalar copies PSUM->SBUF).
  Softmax: the first two phase-2 odd-chunk transpose groups run on the
           tensor engine while the softmax chain (Vector/Scalar) runs:
           negated row max -> Exp with bias + accumulated row sum ->
           fold beta/Z into rows -> PE-transpose; diagonal blocks add
           the identity during the PSUM->SBUF copy, yielding the block
           rows of M = I + beta*softmax(G)^T in SBUF (bf16).
  Phase 2: per 4-chunk group: PE-transpose the 2 odd chunks (two groups
           ahead of the matmuls, hiding the PSUM->SBUF copy latency),
           8 matmuls psum = a_chunk @ M^T, copy PSUM->SBUF staging
           (alternating Vector/Scalar), DMA out 1 MiB granules.
"""

import sys

import numpy as np

sys.path.insert(0, "/opt/trn_rl_repo")

import concourse.bass as bass  # noqa: E402
import concourse.tile as tile  # noqa: E402
from concourse import bacc, mybir  # noqa: E402
from concourse.masks import make_identity  # noqa: E402

P = 128          # partitions / chunk rows
C = 256          # channels
N = 16384        # H*W rows per sample
NCH = N // P     # 128 row-chunks
DG = 8           # chunks per steady-state input DMA granule (1 MiB)
NB = 8           # batch == cores
F32 = mybir.dt.float32
BF16 = mybir.dt.bfloat16


def build_program() -> bass.Bass:
    nc = bacc.Bacc(None, target_bir_lowering=False)
    x = nc.dram_tensor("x", [N, C], F32, kind="ExternalInput")
    beta = nc.dram_tensor("beta", [1, 1], F32, kind="ExternalInput")
    out = nc.dram_tensor("out", [N, C], F32, kind="ExternalOutput")

    # partition-contiguous views: [p, t, c] = row p*NCH+t, channel c.
    # Each partition covers a contiguous 128 KiB HBM span, so every DMA is
    # one long contiguous run per partition. All stages use this same row
    # permutation consistently; G sums over all rows, so the permutation
    # does not change the result.
    x_v = x.rearrange("(p t) c -> p t c", p=P)
    out_v = out.rearrange("(p t) c -> p t c", p=P)

    with tile.TileContext(nc) as tc:
        with (
            tc.tile_pool(name="singles", bufs=1) as singles,
            tc.tile_pool(name="xg", bufs=6) as xg_pool,
            tc.tile_pool(name="att", bufs=1) as att_pool,
            tc.tile_pool(name="stat", bufs=2) as stat_pool,
            tc.tile_pool(name="aTs", bufs=4) as aTs_pool,
            tc.tile_pool(name="stage", bufs=4) as stage_pool,
        ):
            # input granules: tiny head for fast pipeline prime, 1 MiB
            # steady state, tiny tail so G closes right behind the last
            # arrival.  dma_start costs the issuing engine ~0.65us of
            # descriptor generation regardless of size, so the issues are
            # spread over idle queues (sync/gpsimd/tensor) to ramp fast.
            in_granules = (
                [(0, 1), (1, 1), (2, 2), (4, 4)]
                + [(s, DG) for s in range(DG, NCH - DG, DG)]
                + [(120, 4), (124, 2), (126, 1), (127, 1)]
            )
            head_q = [nc.sync, nc.gpsimd, nc.sync, nc.scalar]
            ident = beta_sb = None
            xg_of = {}  # granule start -> landing tile
            for gi, (s, sz) in enumerate(in_granules):
                xgt = xg_pool.tile([P, DG, C], F32, tag="xg", name=f"xg{s}")
                eng = head_q[gi] if gi < 4 else (nc.sync if gi % 2 else nc.gpsimd)
                eng.dma_start(out=xgt[:, :sz, :], in_=x_v[:, s : s + sz, :])
                xg_of[s] = xgt
                if gi == 1:
                    # setup while the first granules are in flight
                    ident = singles.tile([P, P], BF16, tag="ident")
                    make_identity(nc, ident)
                    beta_sb = singles.tile([P, 1], F32, tag="beta")
                    nc.gpsimd.dma_start(
                        out=beta_sb, in_=beta[:].to_broadcast((P, 1))
                    )

            # residents: bf16 x (all chunks), a^T for even chunks < TTAIL.
            # The tail chunks skip phase-1 transposes entirely so the last
            # G matmuls are not queued behind transpose-PSUM recycling.
            TTAIL = 112
            xb_res = singles.tile([P, NCH, C], BF16, tag="xb")
            aT_e = singles.tile([P, TTAIL // 2, 2, P], BF16, tag="aTe")

            att_t = [
                att_pool.tile([P, C], BF16, tag=f"attT{k}", name=f"attT{k}")
                for k in range(2)
            ]

            # tp2/aTs hold phase-2 odd-chunk transposes; tp2 is open across
            # the phase-1 scope so the first groups run in the softmax gap.
            with tc.tile_pool(name="tp2p", bufs=2, space="PSUM") as tp2_pool:
                aTs_of = {}

                def emit_tr(q):
                    # chunks of group q needing a phase-2 transpose: odds
                    # always, plus evens for tail groups (>= TTAIL)
                    chks = [
                        i
                        for i in range(4 * q, 4 * q + 4)
                        if i % 2 == 1 or i >= TTAIL
                    ]
                    nw = len(chks)
                    tps2 = tp2_pool.tile(
                        [P, 4, 2, P], BF16, tag="tp2", name=f"tp2_{q}"
                    )
                    for w, i in enumerate(chks):
                        xb = xb_res[:, i, :]
                        for j in range(2):
                            nc.tensor.transpose(
                                tps2[:, w, j, :], xb[:, j * P : (j + 1) * P], ident
                            )
                    aTs = aTs_pool.tile(
                        [P, 4, 2, P], BF16, tag="aTs", name=f"aTs{q}"
                    )
                    # opposite engine from the group's out-copy, so the
                    # copy engines stay balanced in phase 2
                    if q % 2 == 0:
                        nc.scalar.copy(
                            out=aTs[:, :nw, :, :], in_=tps2[:, :nw, :, :]
                        )
                    else:
                        nc.vector.tensor_copy(
                            out=aTs[:, :nw, :, :], in_=tps2[:, :nw, :, :]
                        )
                    aTs_of[q] = {i: aTs[:, w, :, :] for w, i in enumerate(chks)}

                with (
                    tc.tile_pool(name="gps", bufs=1, space="PSUM") as gps_pool,
                    tc.tile_pool(name="tps", bufs=3, space="PSUM") as tps_pool,
                ):
                    # ---- Phase 1: G = a^T a over all 128 chunks; even
                    #      chunks also PE-transposed into resident a^T ----
                    G = [
                        gps_pool.tile([P, C], F32, tag=f"G{j}", name=f"G{j}")
                        for j in range(2)
                    ]
                    tps_open = {}  # even-pair index m -> psum tile
                    for s, sz in in_granules:
                        nc.vector.tensor_copy(
                            out=xb_res[:, s : s + sz, :], in_=xg_of[s][:, :sz, :]
                        )
                        for i in range(s, s + sz):
                            xb = xb_res[:, i, :]
                            if i % 2 == 0 and i < TTAIL:
                                m, slot = i // 4, (i % 4) // 2
                                if slot == 0:
                                    tps_open[m] = tps_pool.tile(
                                        [P, 2, 2, P], BF16, tag="tp", name=f"tp{m}"
                                    )
                                for j in range(2):
                                    nc.tensor.transpose(
                                        tps_open[m][:, slot, j, :],
                                        xb[:, j * P : (j + 1) * P],
                                        ident,
                                    )
                            for j in range(2):
                                nc.tensor.matmul(
                                    G[j][:],
                                    lhsT=xb[:, j * P : (j + 1) * P],
                                    rhs=xb,
                                    start=(i == 0),
                                    stop=(i == NCH - 1),
                                )
                            if i % 4 == 2 and i < TTAIL:  # pair m complete
                                m = i // 4
                                nc.scalar.copy(
                                    out=aT_e[:, 2 * m : 2 * m + 2, :, :],
                                    in_=tps_open.pop(m)[:],
                                )

                    # tensor fills the softmax wait with phase-2 transposes
                    emit_tr(0)
                    emit_tr(1)

                    # ---- Softmax rows of G scaled by beta, transposed,
                    #      plus I: att_t[k] = block-row k of M ----
                    for j in range(2):
                        nmax = stat_pool.tile(
                            [P, 1], F32, tag="nmax", name=f"nmax{j}"
                        )
                        nc.vector.reduce_max(
                            out=nmax,
                            in_=G[j][:],
                            axis=mybir.AxisListType.X,
                            negate=True,
                        )
                        attj = att_pool.tile(
                            [P, C], BF16, tag=f"att{j}", name=f"att{j}"
                        )
                        zsum = stat_pool.tile(
                            [P, 1], F32, tag="zsum", name=f"zsum{j}"
                        )
                        nc.scalar.activation(
                            out=attj,
                            in_=G[j][:],
                            func=mybir.ActivationFunctionType.Exp,
                            bias=nmax,
                            scale=1.0,
                            accum_out=zsum,
                        )
                        scl = stat_pool.tile([P, 1], F32, tag="scl", name=f"scl{j}")
                        nc.vector.reciprocal(out=scl, in_=zsum)
                        nc.vector.tensor_mul(out=scl, in0=scl, in1=beta_sb)
                        nc.vector.tensor_scalar_mul(out=attj, in0=attj, scalar1=scl)
                        for k in range(2):
                            tpa = tps_pool.tile(
                                [P, 2, 2, P], BF16, tag="tp", name=f"tpa{j}{k}"
                            )
                            nc.tensor.transpose(
                                tpa[:, 0, 0, :], attj[:, k * P : (k + 1) * P], ident
                            )
                            dst = att_t[k][:, j * P : (j + 1) * P]
                            if k == j:
                                nc.vector.tensor_add(
                                    out=dst, in0=tpa[:, 0, 0, :], in1=ident
                                )
                            else:
                                nc.scalar.copy(out=dst, in_=tpa[:, 0, 0, :])

                # ---- Phase 2: out = a @ M^T, streamed out in 1 MiB
                #      granules. Group q = chunks 4q..4q+3; odd chunks
                #      transposed two groups ahead of the matmuls. ----
                NG = NCH // 4  # 32 groups
                with tc.tile_pool(name="ops", bufs=3, space="PSUM") as ops_pool:
                    emit_tr(2)
                    stage = None
                    for q in range(NG):
                        if q + 3 < NG:
                            emit_tr(q + 3)
                        half = q % 2
                        if half == 0:
                            stage = stage_pool.tile(
                                [P, 2 * 4, C], F32, tag="st", name=f"st{q}"
                            )
                        ops = ops_pool.tile([P, 4, C], F32, tag="op", name=f"op{q}")
                        aTs = aTs_of.pop(q)
                        for u in range(4):  # chunks 4q+u
                            i = 4 * q + u
                            if i in aTs:
                                aT_u = aTs[i]
                            else:
                                aT_u = aT_e[:, 2 * q + u // 2, :, :]
                            for j in range(2):
                                nc.tensor.matmul(
                                    ops[:, u, :],
                                    lhsT=aT_u[:, j, :],
                                    rhs=att_t[j][:],
                                    start=(j == 0),
                                    stop=(j == 1),
                                )
                        dst = stage[:, half * 4 : half * 4 + 4, :]
                        if q < 2:
                            # split the copy across both engines to cut the
                            # first-output latency, then 0.5 MiB DMAs
                            nc.vector.tensor_copy(
                                out=dst[:, :2, :], in_=ops[:, :2, :]
                            )
                            nc.scalar.copy(out=dst[:, 2:, :], in_=ops[:, 2:, :])
                            (nc.sync if q == 0 else nc.gpsimd).dma_start(
                                out=out_v[:, 4 * q : 4 * q + 4, :], in_=dst
                            )
                            continue
                        if q % 2 == 0:
                            nc.vector.tensor_copy(out=dst, in_=ops[:])
                        else:
                            nc.scalar.copy(out=dst, in_=ops[:])
                        if half == 1:  # steady state: 1 MiB granules
                            # alternate issue queues so descriptor
                            # generation overlaps the previous drain
                            eng = nc.sync if (q // 2) % 2 == 0 else nc.gpsimd
                            eng.dma_start(
                                out=out_v[:, 8 * (q // 2) : 8 * (q // 2) + 8, :],
                                in_=stage[:],
                            )
    nc.finalize()
    return nc


_NC_CACHE = None


def _get_program() -> bass.Bass:
    global _NC_CACHE
    if _NC_CACHE is None:
        _NC_CACHE = build_program()
    return _NC_CACHE


def run(x: np.ndarray, beta: np.ndarray, trace: bool = False):
    """Shard over 8 cores, run, gather. Returns (out [8,128,128,256], results)."""
    from concourse.bass_utils import run_bass_kernel_spmd

    x = np.ascontiguousarray(np.asarray(x, dtype=np.float32)).reshape(NB, N, C)
    beta_arr = np.ascontiguousarray(
        np.asarray(beta, dtype=np.float32).reshape(1, 1)
    )
    nc = _get_program()
    in_maps = [{"x": x[b], "beta": beta_arr} for b in range(NB)]
    res = run_bass_kernel_spmd(nc, in_maps, core_ids=list(range(NB)), trace=trace)
    outs = np.stack(
        [res.results[b]["out"].reshape(P, P, C) for b in range(NB)], axis=0
    )
    return outs, res


def kernel(**inputs) -> np.ndarray:
    x = np.asarray(inputs["x"])
    beta = np.asarray(inputs["beta"])
    outs, _ = run(x, beta, trace=False)
    return outs.astype(np.float32, copy=False)


# revision 25
# speedup vs baseline: 1.1300x; 1.1300x over previous
"""Channel-attention kernel for Trainium2 (Bass/Tile), 8-core data parallel.

Computes, per batch sample b (x: [B=8, H=128, W=128, C=256] fp32):
    a   = x[b].reshape(N=16384, C)
    G   = a^T @ a                      # [C, C]
    att = softmax(G, axis=-1)
    out = x[b] + beta * (a @ att^T)

Sharding: pure data parallel, one sample per NeuronCore (B == n_cores == 8).

Design notes (v6):
  * The residual add is fused into the second matmul via
    M = I + beta * softmax(G)^T, so phase 2 is pure matmul with no
    elementwise residual pass (bf16 rounding of x through the a @ I path
    is ~2^-9 relative, far inside the 2e-2 gate).
  * dma_start costs the issuing engine ~0.65us of descriptor generation
    (hardware-DGE engines: sync/scalar only; gpsimd's software DGE is an
    order of magnitude slower).  Input granules are 0.5-1 MiB; chunks
    120..127 are fetched FIRST (during the issue-limited ramp) so the
    stream ends with full-size granules and G closes right behind the
    last arrival.
  * a^T transposes: chunks {120..127, 0..87} transpose in phase 1
    (tensor has slack under the input-DMA pace), chunks 88..119 (the
    last five granules) in phase 2, keeping both phases' tensor load
    under the DMA roofline.  a^T is a uniform resident [P, 128, 2, P].

Per-core pipeline (N=16384 rows as 128 chunks of 128 rows, in the
partition-contiguous permutation so every HBM DMA is one contiguous run
per partition):
  Phase 1: stream x granules into a landing pool, cast each granule to
           bf16 (transient pool; resident buffer for chunks 88..119),
           accumulate G = sum_i xb_i^T @ xb_i in 2 PSUM banks,
           PE-transpose phase-1 chunks into resident a^T (Scalar copies
           PSUM->SBUF in pairs).
  Softmax: per 128-row half of G: negated row max -> Exp with bias +
           accumulated row sum (ScalarE) -> fold beta/Z into rows ->
           PE-transpose; diagonal blocks add the identity during the
           PSUM->SBUF copy, yielding block rows of M in SBUF (bf16).
  Phase 2: per 4-chunk group: 8 matmuls psum = a_chunk @ M^T, copy
           PSUM->SBUF staging (alternating Vector/Scalar), DMA out
           1 MiB granules from the sync queue; the chunk-88..119
           transposes are interleaved early, paced by the DMA slack.
"""

import sys

import numpy as np

sys.path.insert(0, "/opt/trn_rl_repo")

import concourse.bass as bass  # noqa: E402
import concourse.tile as tile  # noqa: E402
from concourse import bacc, mybir  # noqa: E402
from concourse.masks import make_identity  # noqa: E402

P = 128          # partitions / chunk rows
C = 256          # channels
N = 16384        # H*W rows per sample
NCH = N // P     # 128 row-chunks
DG = 8           # chunks per steady-state input DMA granule (1 MiB)
NB = 8           # batch == cores
LATE0 = 88       # chunks [LATE0, 120) transpose in phase 2
LATE1 = 120
F32 = mybir.dt.float32
BF16 = mybir.dt.bfloat16


def build_program() -> bass.Bass:
    nc = bacc.Bacc(None, target_bir_lowering=False)
    x = nc.dram_tensor("x", [N, C], F32, kind="ExternalInput")
    beta = nc.dram_tensor("beta", [1, 1], F32, kind="ExternalInput")
    out = nc.dram_tensor("out", [N, C], F32, kind="ExternalOutput")

    # partition-contiguous views: [p, t, c] = row p*NCH+t, channel c.
    # Each partition covers a contiguous 128 KiB HBM span, so every DMA is
    # one long contiguous run per partition. All stages use this same row
    # permutation consistently; G sums over all rows, so the permutation
    # does not change the result.
    x_v = x.rearrange("(p t) c -> p t c", p=P)
    out_v = out.rearrange("(p t) c -> p t c", p=P)

    # arrival order: 120..127 first (ramp), 0..87 steady, 88..119 last
    in_granules = (
        [(120, 4), (124, 4)]
        + [(s, DG) for s in range(0, LATE0, DG)]
        + [(s, DG) for s in range(LATE0, 112, DG)]
        + [(112, 4), (116, 4)]
    )

    def is_late(i):  # phase-2-transposed chunks
        return LATE0 <= i < LATE1

    with tile.TileContext(nc) as tc:
        with (
            tc.tile_pool(name="singles", bufs=1) as singles,
            tc.tile_pool(name="xg", bufs=7) as xg_pool,
            tc.tile_pool(name="xbp", bufs=3) as xb_pool,
            tc.tile_pool(name="att", bufs=1) as att_pool,
            tc.tile_pool(name="stat", bufs=2) as stat_pool,
            tc.tile_pool(name="stage", bufs=4) as stage_pool,
        ):
            ident = beta_sb = None
            xg_of = {}  # granule start -> landing tile
            for gi, (s, sz) in enumerate(in_granules):
                xgt = xg_pool.tile([P, DG, C], F32, tag="xg", name=f"xg{s}")
                eng = nc.scalar if gi == 1 else nc.sync
                eng.dma_start(out=xgt[:, :sz, :], in_=x_v[:, s : s + sz, :])
                xg_of[s] = xgt
                if gi == 1:
                    # setup while the first granules are in flight
                    ident = singles.tile([P, P], BF16, tag="ident")
                    make_identity(nc, ident)
                    beta_sb = singles.tile([P, 1], F32, tag="beta")
                    nc.gpsimd.dma_start(
                        out=beta_sb, in_=beta[:].to_broadcast((P, 1))
                    )

            # residents: a^T for all chunks; bf16 x only for late chunks
            aT = singles.tile([P, NCH, 2, P], BF16, tag="aT")
            xb_late = singles.tile([P, LATE1 - LATE0, C], BF16, tag="xbl")

            att_t = [
                att_pool.tile([P, C], BF16, tag=f"attT{k}", name=f"attT{k}")
                for k in range(2)
            ]

            with tc.tile_pool(name="tp2p", bufs=2, space="PSUM") as tp2_pool:

                def emit_tr4(i0, src):
                    # transpose chunks i0..i0+3 from src into aT slots
                    tps2 = tp2_pool.tile(
                        [P, 4, 2, P], BF16, tag="tp2", name=f"tp2_{i0}"
                    )
                    for w in range(4):
                        xb = src[:, w, :]
                        for j in range(2):
                            nc.tensor.transpose(
                                tps2[:, w, j, :], xb[:, j * P : (j + 1) * P], ident
                            )
                    eng = nc.scalar if (i0 // 4) % 2 == 0 else nc.vector
                    dst = aT[:, i0 : i0 + 4, :, :]
                    if eng is nc.scalar:
                        nc.scalar.copy(out=dst, in_=tps2[:])
                    else:
                        nc.vector.tensor_copy(out=dst, in_=tps2[:])

                with (
                    tc.tile_pool(name="gps", bufs=1, space="PSUM") as gps_pool,
                    tc.tile_pool(name="tps", bufs=3, space="PSUM") as tps_pool,
                ):
                    # ---- Phase 1: G = a^T a over all 128 chunks (in
                    #      arrival order); phase-1 chunks PE-transposed ----
                    G = [
                        gps_pool.tile([P, C], F32, tag=f"G{j}", name=f"G{j}")
                        for j in range(2)
                    ]
                    first_mm, tps_open = True, {}
                    n_mm = 0
                    for s, sz in in_granules:
                        if is_late(s):
                            xb_g = xb_late[:, s - LATE0 : s - LATE0 + sz, :]
                        else:
                            xbt = xb_pool.tile(
                                [P, DG, C], BF16, tag="xb", name=f"xb{s}"
                            )
                            xb_g = xbt[:, :sz, :]
                        nc.vector.tensor_copy(out=xb_g, in_=xg_of[s][:, :sz, :])
                        for u in range(sz):
                            i = s + u
                            xb = xb_g[:, u, :]
                            if not is_late(i):
                                m, slot = i // 4, i % 4
                                if slot == 0:
                                    tps_open[m] = tps_pool.tile(
                                        [P, 4, 2, P], BF16, tag="tp", name=f"tp{m}"
                                    )
                                for j in range(2):
                                    nc.tensor.transpose(
                                        tps_open[m][:, slot, j, :],
                                        xb[:, j * P : (j + 1) * P],
                                        ident,
                                    )
                            n_mm += 1
                            for j in range(2):
                                nc.tensor.matmul(
                                    G[j][:],
                                    lhsT=xb[:, j * P : (j + 1) * P],
                                    rhs=xb,
                                    start=first_mm,
                                    stop=(n_mm == NCH),
                                )
                            first_mm = False
                            if i % 4 == 3 and not is_late(i):  # quad done
                                m = i // 4
                                dst = aT[:, 4 * m : 4 * m + 4, :, :]
                                src = tps_open.pop(m)[:]
                                if m % 2 == 0:
                                    nc.scalar.copy(out=dst, in_=src)
                                else:
                                    nc.vector.tensor_copy(out=dst, in_=src)

                    # ---- Softmax rows of G scaled by beta, transposed,
                    #      plus I: att_t[k] = block-row k of M ----
                    for j in range(2):
                        nmax = stat_pool.tile(
                            [P, 1], F32, tag="nmax", name=f"nmax{j}"
                        )
                        nc.vector.reduce_max(
                            out=nmax,
                            in_=G[j][:],
                            axis=mybir.AxisListType.X,
                            negate=True,
                        )
                        attj = att_pool.tile(
                            [P, C], BF16, tag=f"att{j}", name=f"att{j}"
                        )
                        zsum = stat_pool.tile(
                            [P, 1], F32, tag="zsum", name=f"zsum{j}"
                        )
                        nc.scalar.activation(
                            out=attj,
                            in_=G[j][:],
                            func=mybir.ActivationFunctionType.Exp,
                            bias=nmax,
                            scale=1.0,
                            accum_out=zsum,
                        )
                        scl = stat_pool.tile([P, 1], F32, tag="scl", name=f"scl{j}")
                        nc.vector.reciprocal(out=scl, in_=zsum)
                        nc.vector.tensor_mul(out=scl, in0=scl, in1=beta_sb)
                        nc.vector.tensor_scalar_mul(out=attj, in0=attj, scalar1=scl)
                        for k in range(2):
                            tpa = tps_pool.tile(
                                [P, 2, 2, P], BF16, tag="tp", name=f"tpa{j}{k}"
                            )
                            nc.tensor.transpose(
                                tpa[:, 0, 0, :], attj[:, k * P : (k + 1) * P], ident
                            )
                            dst = att_t[k][:, j * P : (j + 1) * P]
                            if k == j:
                                nc.vector.tensor_add(
                                    out=dst, in0=tpa[:, 0, 0, :], in1=ident
                                )
                            else:
                                nc.scalar.copy(out=dst, in_=tpa[:, 0, 0, :])

                # ---- Phase 2: out = a @ M^T, streamed out in 1 MiB
                #      granules.  Group q = chunks 4q..4q+3.  The late
                #      transposes (chunks 88..119) interleave early,
                #      paced by the DMA slack. ----
                NG = NCH // 4  # 32 groups
                late_trs = list(range(LATE0, LATE1, 4))
                with tc.tile_pool(name="ops", bufs=3, space="PSUM") as ops_pool:
                    for k in range(2):  # first late transposes up front
                        i0 = late_trs.pop(0)
                        emit_tr4(i0, xb_late[:, i0 - LATE0 : i0 - LATE0 + 4, :])
                    stage = None
                    for q in range(NG):
                        if late_trs and q % 2 == 0:
                            i0 = late_trs.pop(0)
                            emit_tr4(
                                i0, xb_late[:, i0 - LATE0 : i0 - LATE0 + 4, :]
                            )
                        half = q % 2
                        if half == 0:
                            stage = stage_pool.tile(
                                [P, 2 * 4, C], F32, tag="st", name=f"st{q}"
                            )
                        ops = ops_pool.tile([P, 4, C], F32, tag="op", name=f"op{q}")
                        for u in range(4):  # chunks 4q+u
                            i = 4 * q + u
                            for j in range(2):
                                nc.tensor.matmul(
                                    ops[:, u, :],
                                    lhsT=aT[:, i, j, :],
                                    rhs=att_t[j][:],
                                    start=(j == 0),
                                    stop=(j == 1),
                                )
                        dst = stage[:, half * 4 : half * 4 + 4, :]
                        if q < 2:
                            # split the copy across both engines to cut the
                            # first-output latency, then 0.5 MiB DMAs
                            nc.vector.tensor_copy(
                                out=dst[:, :2, :], in_=ops[:, :2, :]
                            )
                            nc.scalar.copy(out=dst[:, 2:, :], in_=ops[:, 2:, :])
                            nc.sync.dma_start(
                                out=out_v[:, 4 * q : 4 * q + 4, :], in_=dst
                            )
                            continue
                        if q % 2 == 0:
                            nc.vector.tensor_copy(out=dst, in_=ops[:])
                        else:
                            nc.scalar.copy(out=dst, in_=ops[:])
                        if half == 1:  # steady state: 1 MiB granules
                            nc.sync.dma_start(
                                out=out_v[:, 8 * (q // 2) : 8 * (q // 2) + 8, :],
                                in_=stage[:],
                            )
    nc.finalize()
    return nc


_NC_CACHE = None


def _get_program() -> bass.Bass:
    global _NC_CACHE
    if _NC_CACHE is None:
        _NC_CACHE = build_program()
    return _NC_CACHE


def run(x: np.ndarray, beta: np.ndarray, trace: bool = False):
    """Shard over 8 cores, run, gather. Returns (out [8,128,128,256], results)."""
    from concourse.bass_utils import run_bass_kernel_spmd

    x = np.ascontiguousarray(np.asarray(x, dtype=np.float32)).reshape(NB, N, C)
    beta_arr = np.ascontiguousarray(
        np.asarray(beta, dtype=np.float32).reshape(1, 1)
    )
    nc = _get_program()
    in_maps = [{"x": x[b], "beta": beta_arr} for b in range(NB)]
    res = run_bass_kernel_spmd(nc, in_maps, core_ids=list(range(NB)), trace=trace)
    outs = np.stack(
        [res.results[b]["out"].reshape(P, P, C) for b in range(NB)], axis=0
    )
    return outs, res


def kernel(**inputs) -> np.ndarray:
    x = np.asarray(inputs["x"])
    beta = np.asarray(inputs["beta"])
    outs, _ = run(x, beta, trace=False)
    return outs.astype(np.float32, copy=False)
